# revision 5
# baseline (speedup 1.0000x reference)
"""TRN2 Bass kernel for nn_COV_75359496176097.

reference():
    B2 = B[0]                               # (8192, 8192)
    rn = sqrt(1 / sum(B2*B2, axis=1))       # row norms
    A  = rn * B2 * exp(tile(logstd, 64))[:, None]
    samples = tile(mu,64) + einsum('mk,bk->bm', A, eps[:,:,0])
    returns (mu_out, logvar, samples), each (128, 64, 128)

Strategy: shard B by rows across 8 cores (1024 rows each, no collectives),
all data in bf16 (tolerance 2e-2 >> bf16's ~3e-3 dot error) - halves HBM
traffic vs fp32: ~19.4MB/core. The whole packed input (64 k-tiles of
[B^T | eps^T] plus replicated exp(logstd)/mu rows, 148KB/partition) sits
resident in SBUF - no slot recycling. The DMA stream is issued as ~1.2MB
chunks alternating between the sync HWDGE queue and the gpsimd SWDGE
queue so two descriptor generators feed the 16 SDMA engines in parallel;
each chunk has its own completion semaphore (cross-queue completion is
not FIFO).

Per k-tile the PE runs two bf16 acc matmuls (eps^T stationary, B^T
moving, PSUM-accumulated). Row norms: DVE squares low columns, ACT high
columns (fused 2-tile ops via 3D strided APs to amortize per-op fixed
cost); for 'paired' pairs DVE also sums the two square-tiles so the
ones-stationary norm matmul runs once per pair instead of per tile -
trading cheap DVE adds for expensive PE moving columns. The pairing
fraction balances PE vs DVE vs ACT, all under the DMA roofline. The last
pair is unpaired and split per-tile so only tile 63's square chain
remains after the final chunk lands.

Epilogue: out = (acc*els) * sqrt(1/nrm) + mu. DVE reciprocal_approx_fast
on PSUM nrm (fp32), ACT sqrt -> bf16, then bf16 mul/add at DVE 2x rate;
bf16 output DMA per column half. Square and Sqrt share one ACT table set
(loaded once by a dummy sqrt during warmup). A 56-matmul warmup keeps
the PE HAM clock monitor warm until tile 0 lands.

Raw Bass (not Tile): one semaphore wait per instruction; consecutive
standalone waits AND together. Norm matmuls run one pair behind the
square producers (4 sq buffers -> 3 pairs of slack) so DVE/ACT never
stall the PE.
"""

import sys
from contextlib import ExitStack

if "/opt/trn_rl_repo" not in sys.path:
    sys.path.insert(0, "/opt/trn_rl_repo")

import ml_dtypes
import numpy as np

import concourse.bacc as bacc
import concourse.mybir as mybir
from concourse import bass_utils

Z = 128
NS = 64
M = Z * NS          # 8192
BATCH = 128
NCORES = 8
RPC = M // NCORES   # 1024 rows of B per core
KT = M // 128       # 64 k-tiles
TW = RPC + BATCH    # 1152 packed tile width (B^T cols | eps^T cols)
TCOLS = KT * TW     # 73728 tile columns
NTC = TCOLS + 2 * RPC  # + els, mu replicated rows -> 75776 total sbuf cols
NPAIR = KT // 2
WD_P = 704          # DVE/ACT column split for paired pairs
WD_U = 512          # and for unpaired (must be 512: single producer per half)
NSQ = 4             # square-buffer slots
NWARM = 56

# DMA chunks, in tiles; 'EM' is the els/mu constants chunk. Odd indices go
# to the gpsimd SWDGE queue, even to the sync HWDGE queue.
CHUNKS = [
    (0, 2), (2, 4), (4, 8), "EM",
    (8, 12), (12, 16), (16, 20), (20, 24), (24, 28), (28, 32),
    (32, 36), (36, 40), (40, 44), (44, 48), (48, 52), (52, 56),
    (56, 58), (58, 60), (60, 62), (62, 63), (63, 64),
]
NDMA = len(CHUNKS)
EM_IDX = CHUNKS.index("EM")


def _tile_dma_idx(t):
    for i, ch in enumerate(CHUNKS):
        if ch != "EM" and ch[0] <= t < ch[1]:
            return i
    raise AssertionError(t)


def _same_chunk(q):
    return _tile_dma_idx(2 * q) == _tile_dma_idx(2 * q + 1)


def _paired(q):
    return (q % 8) < 5 and _same_chunk(q)


f32 = mybir.dt.float32
bf16 = mybir.dt.bfloat16

_nc_cache = {}


def _build():
    nc = bacc.Bacc("TRN2", debug=False)

    bte_d = nc.dram_tensor("bte", (128, NTC), bf16, kind="ExternalInput")
    out_d = nc.dram_tensor("out", (BATCH, RPC), bf16, kind="ExternalOutput")

    # per-pair semaphore thresholds, precomputed (python-side counters)
    d0c = d1c = actc = 0
    thr = {}  # q -> dict with thresholds
    for q in range(NPAIR):
        if _paired(q):
            d0c += 1
            actc += 1
            d1c += 1
            thr[q] = dict(p=True, h0=d0c, h1=d1c, act=actc)
        elif _same_chunk(q):
            d0c += 1
            d1c += 1
            thr[q] = dict(p=False, eh0=d0c, eh1=d1c, oh0=d0c, oh1=d1c)
        else:
            d0c += 2
            d1c += 2
            thr[q] = dict(p=False, eh0=d0c - 1, eh1=d1c - 1, oh0=d0c, oh1=d1c)

    with ExitStack() as ctx:
        e = ctx.enter_context
        bte = e(nc.sbuf_tensor("bte_sb", [128, NTC], bf16))
        # slot s holds pair q=s (mod NSQ): [:, 0, :] even tile, [:, 1, :] odd
        sq2 = [e(nc.sbuf_tensor(f"sq2_{i}", [128, 2, RPC], bf16)) for i in range(NSQ)]
        ones = e(nc.sbuf_tensor("ones", [128, 128], bf16))
        dummy = e(nc.sbuf_tensor("dmy_sb", [128, 1], f32))
        scale = e(nc.sbuf_tensor("scale", [128, RPC], bf16))
        rr = e(nc.sbuf_tensor("rr", [128, RPC], f32))
        rs = e(nc.sbuf_tensor("rs", [128, RPC], bf16))
        out_sb = e(nc.sbuf_tensor("out_sb", [128, RPC], bf16))
        acc = e(nc.psum_tensor([128, RPC], f32))
        nrm = e(nc.psum_tensor([128, RPC], f32))
        warm_ps = e(nc.psum_tensor([128, 512], f32))

        s_dma = [e(nc.semaphore(name=f"s_dma{i}")) for i in range(NDMA)]
        s_wm = e(nc.semaphore(name="s_wm"))
        s_act = e(nc.semaphore(name="s_act"))
        s_d0 = e(nc.semaphore(name="s_d0"))
        s_d1 = e(nc.semaphore(name="s_d1"))
        s_nm = e(nc.semaphore(name="s_nm"))
        s_acc = e(nc.semaphore(name="s_acc"))
        s_r = e(nc.semaphore(name="s_r"))
        s_x = e(nc.semaphore(name="s_x"))
        s_out = e(nc.semaphore(name="s_out"))
        s_od = e(nc.semaphore(name="s_od"))

        block = e(nc.Block())

        H = (slice(0, 512), slice(512, RPC))

        def bslice(t, a, b):
            return bte[:, t * TW + a:t * TW + b]

        def bpair(q, a, b):
            """[128, 2, b-a] view of cols [a:b) of tiles (2q, 2q+1)."""
            te = 2 * q
            return bte[:, te * TW:(te + 2) * TW].rearrange(
                "p (t w) -> p t w", t=2
            )[:, :, a:b]

        def chunk_cols(ch):
            if ch == "EM":
                return TCOLS, NTC
            return ch[0] * TW, ch[1] * TW

        @block.sync
        def _(sync):
            for i, ch in enumerate(CHUNKS):
                if i % 2 == 1:
                    continue
                c0, c1 = chunk_cols(ch)
                sync.dma_start(bte[:, c0:c1], bte_d.ap()[:, c0:c1]).then_inc(
                    s_dma[i], 16
                )
            for h in range(2):
                sync.wait_ge(s_out, h + 1)
                sync.dma_start(out_d.ap()[:, H[h]], out_sb[:, H[h]]).then_inc(
                    s_od, 16
                )
            sync.wait_ge(s_od, 32)
            sync.nop()

        @block.gpsimd
        def _(gpsimd):
            for i, ch in enumerate(CHUNKS):
                if i % 2 == 0:
                    continue
                c0, c1 = chunk_cols(ch)
                gpsimd.dma_start(bte[:, c0:c1], bte_d.ap()[:, c0:c1]).then_inc(
                    s_dma[i], 16
                )

        @block.tensor
        def _(tensor):
            # warmup: pin the PE HAM activity monitor warm until tile 0 lands
            tensor.wait_ge(s_wm, 1)
            for _ in range(NWARM):
                nc.tensor.matmul(
                    warm_ps[:, 0:128], ones[:], ones[:], start=True, stop=True
                )

            def norm_mms(q, sub="all"):
                # sub: 'all' (normal, one pair behind), 'e'/'o' for the
                # split last pair
                s = q % NSQ
                tq = thr[q]
                if tq["p"]:
                    qst, qsp = q == 0, q == NPAIR - 1
                    tensor.wait_ge(s_d0, tq["h0"])
                    nc.tensor.matmul(
                        nrm[:, H[0]], ones[:], sq2[s][:, 0, H[0]],
                        start=qst, stop=qsp,
                    )
                    tensor.wait_ge(s_d1, tq["h1"])
                    nc.tensor.matmul(
                        nrm[:, H[1]], ones[:], sq2[s][:, 0, H[1]],
                        start=qst, stop=qsp,
                    ).then_inc(s_nm, 1)
                    return
                tiles = {"all": (0, 1), "e": (0,), "o": (1,)}[sub]
                for ti in tiles:
                    qst = q == 0 and ti == 0
                    qsp = q == NPAIR - 1 and ti == 1
                    tensor.wait_ge(s_d0, tq["eh0"] if ti == 0 else tq["oh0"])
                    nc.tensor.matmul(
                        nrm[:, H[0]], ones[:], sq2[s][:, ti, H[0]],
                        start=qst, stop=qsp,
                    )
                    tensor.wait_ge(s_d1, tq["eh1"] if ti == 0 else tq["oh1"])
                    ins = nc.tensor.matmul(
                        nrm[:, H[1]], ones[:], sq2[s][:, ti, H[1]],
                        start=qst, stop=qsp,
                    )
                    # exactly one s_nm inc per pair-slot: the final sub-call
                    if ti == tiles[-1] and sub != "e":
                        ins.then_inc(s_nm, 1)

            last_q = NPAIR - 1
            last_split = not _same_chunk(last_q)
            seen = -1
            for t in range(KT):
                di = _tile_dma_idx(t)
                if di > seen:
                    if last_split and t == KT - 1:
                        # run the last pair's even-tile norm matmuls before
                        # stalling on the final chunk
                        norm_mms(last_q, sub="e")
                    tensor.wait_ge(s_dma[di], 16)
                    seen = di
                st, sp = t == 0, t == KT - 1
                eps_v = bslice(t, RPC, TW)
                for h in range(2):
                    ins = nc.tensor.matmul(
                        acc[:, H[h]], eps_v, bslice(t, h * 512, (h + 1) * 512),
                        start=st, stop=sp,
                    )
                if sp:
                    ins.then_inc(s_acc, 1)
                if t % 2 == 1 and t >= 3:
                    norm_mms(t // 2 - 1)
            norm_mms(last_q, sub="o" if last_split else "all")

        @block.scalar
        def _(scalar):
            # dummy sqrt first: loads the sqrt_and_others ACT table set
            # (which also contains Square) once, during the DMA fill
            scalar.wait_ge(s_wm, 1)
            nc.scalar.sqrt(dummy[:], ones[:, 0:1])
            seen = -1
            for q in range(NPAIR):
                s = q % NSQ
                tq = thr[q]
                wd = WD_P if tq["p"] else WD_U
                die = _tile_dma_idx(2 * q)
                dio = _tile_dma_idx(2 * q + 1)
                if die > seen:
                    scalar.wait_ge(s_dma[die], 16)
                    seen = die
                if q >= NSQ:
                    scalar.wait_ge(s_nm, q - (NSQ - 1))
                if dio == die:
                    ins = nc.scalar.square(
                        sq2[s][:, :, wd:RPC], bpair(q, wd, RPC)
                    )
                    ins.then_inc(s_act if tq["p"] else s_d1, 1)
                else:
                    nc.scalar.square(
                        sq2[s][:, 0, wd:RPC], bslice(2 * q, wd, RPC)
                    ).then_inc(s_d1, 1)
                    scalar.wait_ge(s_dma[dio], 16)
                    seen = dio
                    nc.scalar.square(
                        sq2[s][:, 1, wd:RPC], bslice(2 * q + 1, wd, RPC)
                    ).then_inc(s_d1, 1)
            for h in range(2):
                scalar.wait_ge(s_r, h + 1)
                nc.scalar.sqrt(rs[:, H[h]], rr[:, H[h]]).then_inc(s_x, 1)

        @block.vector
        def _(vector):
            nc.vector.memset(ones[:], 1.0).then_inc(s_wm, 1)
            seen = -1
            for q in range(NPAIR):
                s = q % NSQ
                tq = thr[q]
                wd = WD_P if tq["p"] else WD_U
                die = _tile_dma_idx(2 * q)
                dio = _tile_dma_idx(2 * q + 1)
                if die > seen:
                    vector.wait_ge(s_dma[die], 16)
                    seen = die
                if q >= NSQ:
                    vector.wait_ge(s_nm, q - (NSQ - 1))
                if dio == die:
                    ins = nc.vector.tensor_mul(
                        sq2[s][:, :, 0:wd], bpair(q, 0, wd), bpair(q, 0, wd)
                    )
                    if not tq["p"]:
                        ins.then_inc(s_d0, 1)
                else:
                    be = bslice(2 * q, 0, wd)
                    nc.vector.tensor_mul(
                        sq2[s][:, 0, 0:wd], be, be
                    ).then_inc(s_d0, 1)
                    vector.wait_ge(s_dma[dio], 16)
                    seen = dio
                    bo = bslice(2 * q + 1, 0, wd)
                    nc.vector.tensor_mul(
                        sq2[s][:, 1, 0:wd], bo, bo
                    ).then_inc(s_d0, 1)
                if tq["p"]:
                    nc.vector.tensor_add(
                        sq2[s][:, 0, H[0]], sq2[s][:, 0, H[0]], sq2[s][:, 1, H[0]]
                    ).then_inc(s_d0, 1)
                    vector.wait_ge(s_act, tq["act"])
                    nc.vector.tensor_add(
                        sq2[s][:, 0, H[1]], sq2[s][:, 0, H[1]], sq2[s][:, 1, H[1]]
                    ).then_inc(s_d1, 1)

            # epilogue: out = (acc*els) * sqrt(1/nrm) + mu, by column halves
            els = bte[:, TCOLS:TCOLS + RPC]
            mu_v = bte[:, TCOLS + RPC:NTC]
            vector.wait_ge(s_dma[EM_IDX], 16)
            vector.wait_ge(s_acc, 1)
            for h in range(2):
                nc.vector.tensor_mul(scale[:, H[h]], acc[:, H[h]], els[:, H[h]])
            vector.wait_ge(s_nm, NPAIR)
            for h in range(2):
                nc.vector.reciprocal_approx_fast(
                    out=rr[:, H[h]], in_=nrm[:, H[h]]
                ).then_inc(s_r, 1)
            for h in range(2):
                vector.wait_ge(s_x, h + 1)
                nc.vector.tensor_mul(out_sb[:, H[h]], scale[:, H[h]], rs[:, H[h]])
                nc.vector.tensor_add(
                    out_sb[:, H[h]], out_sb[:, H[h]], mu_v[:, H[h]]
                ).then_inc(s_out, 1)

    nc.compile()
    return nc


def _get_nc():
    if "nc" not in _nc_cache:
        _nc_cache["nc"] = _build()
    return _nc_cache["nc"]


def _prep_inputs(mu, logstd, B, eps):
    bfl = ml_dtypes.bfloat16
    B2 = B[0]
    Bb = B2.astype(bfl)                                  # (M, M)
    epsT3 = np.ascontiguousarray(eps[:, :, 0].T).astype(bfl).reshape(KT, 128, BATCH)
    mu_rep = np.tile(mu[0], NS)                          # (M,)
    logstd_rep = np.tile(logstd, NS)                     # (M,)
    els_rep = np.exp(logstd_rep).astype(np.float32)      # (M,)

    in_maps = []
    for c in range(NCORES):
        rows = slice(c * RPC, (c + 1) * RPC)
        bt3 = np.ascontiguousarray(Bb[rows, :].T).reshape(KT, 128, RPC)
        tile_block = np.concatenate([bt3, epsT3], axis=2)   # (KT, 128, TW)
        arr = np.empty((128, NTC), dtype=bfl)
        arr[:, 0:TCOLS] = tile_block.transpose(1, 0, 2).reshape(128, TCOLS)
        arr[:, TCOLS:TCOLS + RPC] = np.broadcast_to(
            els_rep[rows].astype(bfl)[None, :], (128, RPC)
        )
        arr[:, TCOLS + RPC:NTC] = np.broadcast_to(
            mu_rep[rows].astype(bfl)[None, :], (128, RPC)
        )
        in_maps.append({"bte": arr})
    return in_maps, mu_rep, logstd_rep


def _run(mu, logstd, B, eps, batch_size, trace=False, trace_kwargs=None):
    mu = np.asarray(mu, dtype=np.float32)
    logstd = np.asarray(logstd, dtype=np.float32)
    B = np.asarray(B, dtype=np.float32)
    eps = np.asarray(eps, dtype=np.float32)
    b = int(batch_size)
    assert B.shape == (1, M, M) and eps.shape == (b, M, 1) and b == BATCH

    in_maps, mu_rep, logstd_rep = _prep_inputs(mu, logstd, B, eps)

    nc = _get_nc()
    kw = {}
    if trace:
        kw = dict(trace=True, trace_cores=list(range(NCORES)))
        if trace_kwargs:
            kw.update(trace_kwargs)
    res = bass_utils.run_bass_kernel_spmd(
        nc, in_maps, core_ids=list(range(NCORES)), **kw
    )

    samples_bm = np.concatenate(
        [np.asarray(res.results[c]["out"]).astype(np.float32) for c in range(NCORES)],
        axis=1,
    )  # (BATCH, M)
    samples = samples_bm.reshape(b, NS, Z)
    mu_out = np.broadcast_to(mu_rep[None, :], (b, M)).reshape(b, NS, Z).copy()
    logvar = (
        np.broadcast_to(2.0 * logstd_rep[None, :], (b, M)).reshape(b, NS, Z).copy()
    )
    return (mu_out, logvar, samples), res


def kernel(mu, logstd, B, eps, batch_size):
    outs, _ = _run(mu, logstd, B, eps, batch_size, trace=False)
    return outs


# revision 8
# speedup vs baseline: 1.0492x; 1.0492x over previous
"""TRN2 Bass kernel for nn_COV_75359496176097.

reference():
    B2 = B[0]                               # (8192, 8192)
    rn = sqrt(1 / sum(B2*B2, axis=1))       # row norms
    A  = rn * B2 * exp(tile(logstd, 64))[:, None]
    samples = tile(mu,64) + einsum('mk,bk->bm', A, eps[:,:,0])
    returns (mu_out, logvar, samples), each (128, 64, 128)

Strategy: shard B by rows across 8 cores (1024 rows each, no collectives),
all data in bf16 (tolerance 2e-2 >> bf16's ~3e-3 dot error) - halves HBM
traffic vs fp32: ~19.4MB/core. The whole packed input (64 k-tiles of
[B^T | eps^T] plus replicated exp(logstd)/mu rows, 148KB/partition) sits
resident in SBUF - no slot recycling. The DMA stream is issued as ~1.2MB
chunks alternating between the sync HWDGE queue and the gpsimd SWDGE
queue so two descriptor generators feed the 16 SDMA engines in parallel;
each chunk has its own completion semaphore (cross-queue completion is
not FIFO).

Per k-tile the PE runs two bf16 acc matmuls (eps^T stationary, B^T
moving, PSUM-accumulated). Row norms: DVE squares low columns, ACT high
columns (fused 2-tile ops via 3D strided APs to amortize per-op fixed
cost); for 'paired' pairs DVE also sums the two square-tiles so the
ones-stationary norm matmul runs once per pair instead of per tile -
trading cheap DVE adds for expensive PE moving columns. The pairing
fraction balances PE vs DVE vs ACT, all under the DMA roofline. The last
pair is unpaired and split per-tile so only tile 63's square chain
remains after the final chunk lands.

Epilogue: out = (acc*els) * sqrt(1/nrm) + mu. DVE reciprocal_approx_fast
on PSUM nrm (fp32), ACT sqrt -> bf16, then bf16 mul/add at DVE 2x rate;
bf16 output DMA per column half. Square and Sqrt share one ACT table set
(loaded once by a dummy sqrt during warmup). A 56-matmul warmup keeps
the PE HAM clock monitor warm until tile 0 lands.

Raw Bass (not Tile): one semaphore wait per instruction; consecutive
standalone waits AND together. Norm matmuls run one pair behind the
square producers (4 sq buffers -> 3 pairs of slack) so DVE/ACT never
stall the PE.
"""

import sys
from contextlib import ExitStack

if "/opt/trn_rl_repo" not in sys.path:
    sys.path.insert(0, "/opt/trn_rl_repo")

import ml_dtypes
import numpy as np

import concourse.bacc as bacc
import concourse.mybir as mybir
from concourse import bass_utils

Z = 128
NS = 64
M = Z * NS          # 8192
BATCH = 128
NCORES = 8
RPC = M // NCORES   # 1024 rows of B per core
KT = M // 128       # 64 k-tiles
TW = RPC + BATCH    # 1152 packed tile width (B^T cols | eps^T cols)
TCOLS = KT * TW     # 73728 tile columns
NTC = TCOLS + 2 * RPC  # + els, mu replicated rows -> 75776 total sbuf cols
NPAIR = KT // 2
WD_P = 704          # DVE/ACT column split for paired pairs
WD_U = 512          # and for unpaired (must be 512: single producer per half)
NSQ = 4             # square-buffer slots
NWARM = 44

# DMA chunks, in tiles; 'EM' is the els/mu constants chunk. All issued in
# order on the sync HWDGE ring (gpsimd SWDGE adds a ~6.5us DRAIN per DMA
# and a ~5us preamble barrier - measured strictly worse).
CHUNKS = [
    (0, 2), (2, 4), (4, 8), "EM",
    (8, 12), (12, 16), (16, 20), (20, 24), (24, 28), (28, 32),
    (32, 36), (36, 40), (40, 44), (44, 48), (48, 52), (52, 56),
    (56, 58), (58, 60), (60, 62), (62, 63), (63, 64),
]
NDMA = len(CHUNKS)
EM_IDX = CHUNKS.index("EM")


def _tile_dma_idx(t):
    for i, ch in enumerate(CHUNKS):
        if ch != "EM" and ch[0] <= t < ch[1]:
            return i
    raise AssertionError(t)


def _same_chunk(q):
    return _tile_dma_idx(2 * q) == _tile_dma_idx(2 * q + 1)


def _paired(q):
    return (q % 8) < 5 and _same_chunk(q)


f32 = mybir.dt.float32
bf16 = mybir.dt.bfloat16

_nc_cache = {}


def _build():
    nc = bacc.Bacc("TRN2", debug=False)

    bte_d = nc.dram_tensor("bte", (128, NTC), bf16, kind="ExternalInput")
    out_d = nc.dram_tensor("out", (BATCH, RPC), bf16, kind="ExternalOutput")

    # per-pair semaphore thresholds, precomputed (python-side counters)
    d0c = d1c = actc = 0
    thr = {}  # q -> dict with thresholds
    for q in range(NPAIR):
        if _paired(q):
            d0c += 1
            actc += 1
            d1c += 1
            thr[q] = dict(p=True, h0=d0c, h1=d1c, act=actc)
        elif _same_chunk(q):
            d0c += 1
            d1c += 1
            thr[q] = dict(p=False, eh0=d0c, eh1=d1c, oh0=d0c, oh1=d1c)
        else:
            d0c += 2
            d1c += 2
            thr[q] = dict(p=False, eh0=d0c - 1, eh1=d1c - 1, oh0=d0c, oh1=d1c)

    with ExitStack() as ctx:
        e = ctx.enter_context
        bte = e(nc.sbuf_tensor("bte_sb", [128, NTC], bf16))
        # slot s holds pair q=s (mod NSQ): [:, 0, :] even tile, [:, 1, :] odd
        sq2 = [e(nc.sbuf_tensor(f"sq2_{i}", [128, 2, RPC], bf16)) for i in range(NSQ)]
        ones = e(nc.sbuf_tensor("ones", [128, 128], bf16))
        dummy = e(nc.sbuf_tensor("dmy_sb", [128, 1], f32))
        scale = e(nc.sbuf_tensor("scale", [128, RPC], bf16))
        rr = e(nc.sbuf_tensor("rr", [128, RPC], f32))
        rs = e(nc.sbuf_tensor("rs", [128, RPC], bf16))
        out_sb = e(nc.sbuf_tensor("out_sb", [128, RPC], bf16))
        acc = e(nc.psum_tensor([128, RPC], f32))
        nrm = e(nc.psum_tensor([128, RPC], f32))
        warm_ps = e(nc.psum_tensor([128, 512], f32))

        s_dma = [e(nc.semaphore(name=f"s_dma{i}")) for i in range(NDMA)]
        s_wm = e(nc.semaphore(name="s_wm"))
        s_act = e(nc.semaphore(name="s_act"))
        s_d0 = e(nc.semaphore(name="s_d0"))
        s_d1 = e(nc.semaphore(name="s_d1"))
        s_nm = e(nc.semaphore(name="s_nm"))
        s_acc = e(nc.semaphore(name="s_acc"))
        s_r = e(nc.semaphore(name="s_r"))
        s_x = e(nc.semaphore(name="s_x"))
        s_out = e(nc.semaphore(name="s_out"))
        s_od = e(nc.semaphore(name="s_od"))

        block = e(nc.Block())

        H = (slice(0, 512), slice(512, RPC))

        def bslice(t, a, b):
            return bte[:, t * TW + a:t * TW + b]

        def bpair(q, a, b):
            """[128, 2, b-a] view of cols [a:b) of tiles (2q, 2q+1)."""
            te = 2 * q
            return bte[:, te * TW:(te + 2) * TW].rearrange(
                "p (t w) -> p t w", t=2
            )[:, :, a:b]

        def chunk_cols(ch):
            if ch == "EM":
                return TCOLS, NTC
            return ch[0] * TW, ch[1] * TW

        @block.sync
        def _(sync):
            for i, ch in enumerate(CHUNKS):
                c0, c1 = chunk_cols(ch)
                sync.dma_start(bte[:, c0:c1], bte_d.ap()[:, c0:c1]).then_inc(
                    s_dma[i], 16
                )
            for h in range(2):
                sync.wait_ge(s_out, h + 1)
                sync.dma_start(out_d.ap()[:, H[h]], out_sb[:, H[h]]).then_inc(
                    s_od, 16
                )
            sync.wait_ge(s_od, 32)
            sync.nop()

        @block.tensor
        def _(tensor):
            # warmup: pin the PE HAM activity monitor warm until tile 0 lands
            tensor.wait_ge(s_wm, 1)
            for _ in range(NWARM):
                nc.tensor.matmul(
                    warm_ps[:, 0:128], ones[:], ones[:], start=True, stop=True
                )

            def norm_mms(q, sub="all"):
                # sub: 'all' (normal, one pair behind), 'e'/'o' for the
                # split last pair
                s = q % NSQ
                tq = thr[q]
                if tq["p"]:
                    qst, qsp = q == 0, q == NPAIR - 1
                    tensor.wait_ge(s_d0, tq["h0"])
                    nc.tensor.matmul(
                        nrm[:, H[0]], ones[:], sq2[s][:, 0, H[0]],
                        start=qst, stop=qsp,
                    )
                    tensor.wait_ge(s_d1, tq["h1"])
                    nc.tensor.matmul(
                        nrm[:, H[1]], ones[:], sq2[s][:, 0, H[1]],
                        start=qst, stop=qsp,
                    ).then_inc(s_nm, 1)
                    return
                tiles = {"all": (0, 1), "e": (0,), "o": (1,)}[sub]
                for ti in tiles:
                    qst = q == 0 and ti == 0
                    qsp = q == NPAIR - 1 and ti == 1
                    tensor.wait_ge(s_d0, tq["eh0"] if ti == 0 else tq["oh0"])
                    nc.tensor.matmul(
                        nrm[:, H[0]], ones[:], sq2[s][:, ti, H[0]],
                        start=qst, stop=qsp,
                    )
                    tensor.wait_ge(s_d1, tq["eh1"] if ti == 0 else tq["oh1"])
                    ins = nc.tensor.matmul(
                        nrm[:, H[1]], ones[:], sq2[s][:, ti, H[1]],
                        start=qst, stop=qsp,
                    )
                    # exactly one s_nm inc per pair-slot: the final sub-call
                    if ti == tiles[-1] and sub != "e":
                        ins.then_inc(s_nm, 1)

            last_q = NPAIR - 1
            last_split = not _same_chunk(last_q)
            seen = -1
            for t in range(KT):
                di = _tile_dma_idx(t)
                if di > seen:
                    if last_split and t == KT - 1:
                        # run the last pair's even-tile norm matmuls before
                        # stalling on the final chunk
                        norm_mms(last_q, sub="e")
                    tensor.wait_ge(s_dma[di], 16)
                    seen = di
                st, sp = t == 0, t == KT - 1
                eps_v = bslice(t, RPC, TW)
                for h in range(2):
                    ins = nc.tensor.matmul(
                        acc[:, H[h]], eps_v, bslice(t, h * 512, (h + 1) * 512),
                        start=st, stop=sp,
                    )
                if sp:
                    ins.then_inc(s_acc, 1)
                if t % 2 == 1 and t >= 3:
                    norm_mms(t // 2 - 1)
            norm_mms(last_q, sub="o" if last_split else "all")

        @block.scalar
        def _(scalar):
            # dummy sqrt first: loads the sqrt_and_others ACT table set
            # (which also contains Square) once, during the DMA fill
            scalar.wait_ge(s_wm, 1)
            nc.scalar.sqrt(dummy[:], ones[:, 0:1])
            seen = -1
            for q in range(NPAIR):
                s = q % NSQ
                tq = thr[q]
                wd = WD_P if tq["p"] else WD_U
                die = _tile_dma_idx(2 * q)
                dio = _tile_dma_idx(2 * q + 1)
                if die > seen:
                    scalar.wait_ge(s_dma[die], 16)
                    seen = die
                if q >= NSQ:
                    scalar.wait_ge(s_nm, q - (NSQ - 1))
                if dio == die:
                    ins = nc.scalar.square(
                        sq2[s][:, :, wd:RPC], bpair(q, wd, RPC)
                    )
                    ins.then_inc(s_act if tq["p"] else s_d1, 1)
                else:
                    nc.scalar.square(
                        sq2[s][:, 0, wd:RPC], bslice(2 * q, wd, RPC)
                    ).then_inc(s_d1, 1)
                    scalar.wait_ge(s_dma[dio], 16)
                    seen = dio
                    nc.scalar.square(
                        sq2[s][:, 1, wd:RPC], bslice(2 * q + 1, wd, RPC)
                    ).then_inc(s_d1, 1)
            for h in range(2):
                scalar.wait_ge(s_r, h + 1)
                nc.scalar.sqrt(rs[:, H[h]], rr[:, H[h]]).then_inc(s_x, 1)

        @block.vector
        def _(vector):
            nc.vector.memset(ones[:], 1.0).then_inc(s_wm, 1)
            seen = -1
            for q in range(NPAIR):
                s = q % NSQ
                tq = thr[q]
                wd = WD_P if tq["p"] else WD_U
                die = _tile_dma_idx(2 * q)
                dio = _tile_dma_idx(2 * q + 1)
                if die > seen:
                    vector.wait_ge(s_dma[die], 16)
                    seen = die
                if q >= NSQ:
                    vector.wait_ge(s_nm, q - (NSQ - 1))
                if dio == die:
                    ins = nc.vector.tensor_mul(
                        sq2[s][:, :, 0:wd], bpair(q, 0, wd), bpair(q, 0, wd)
                    )
                    if not tq["p"]:
                        ins.then_inc(s_d0, 1)
                else:
                    be = bslice(2 * q, 0, wd)
                    nc.vector.tensor_mul(
                        sq2[s][:, 0, 0:wd], be, be
                    ).then_inc(s_d0, 1)
                    vector.wait_ge(s_dma[dio], 16)
                    seen = dio
                    bo = bslice(2 * q + 1, 0, wd)
                    nc.vector.tensor_mul(
                        sq2[s][:, 1, 0:wd], bo, bo
                    ).then_inc(s_d0, 1)
                if tq["p"]:
                    nc.vector.tensor_add(
                        sq2[s][:, 0, H[0]], sq2[s][:, 0, H[0]], sq2[s][:, 1, H[0]]
                    ).then_inc(s_d0, 1)
                    vector.wait_ge(s_act, tq["act"])
                    nc.vector.tensor_add(
                        sq2[s][:, 0, H[1]], sq2[s][:, 0, H[1]], sq2[s][:, 1, H[1]]
                    ).then_inc(s_d1, 1)

            # epilogue: out = (acc*els) * sqrt(1/nrm) + mu, by column halves
            els = bte[:, TCOLS:TCOLS + RPC]
            mu_v = bte[:, TCOLS + RPC:NTC]
            vector.wait_ge(s_dma[EM_IDX], 16)
            vector.wait_ge(s_acc, 1)
            for h in range(2):
                nc.vector.tensor_mul(scale[:, H[h]], acc[:, H[h]], els[:, H[h]])
            vector.wait_ge(s_nm, NPAIR)
            for h in range(2):
                nc.vector.reciprocal_approx_fast(
                    out=rr[:, H[h]], in_=nrm[:, H[h]]
                ).then_inc(s_r, 1)
            for h in range(2):
                vector.wait_ge(s_x, h + 1)
                nc.vector.tensor_mul(out_sb[:, H[h]], scale[:, H[h]], rs[:, H[h]])
                nc.vector.tensor_add(
                    out_sb[:, H[h]], out_sb[:, H[h]], mu_v[:, H[h]]
                ).then_inc(s_out, 1)

    nc.compile()
    return nc


def _get_nc():
    if "nc" not in _nc_cache:
        _nc_cache["nc"] = _build()
    return _nc_cache["nc"]


def _prep_inputs(mu, logstd, B, eps):
    bfl = ml_dtypes.bfloat16
    B2 = B[0]
    Bb = B2.astype(bfl)                                  # (M, M)
    epsT3 = np.ascontiguousarray(eps[:, :, 0].T).astype(bfl).reshape(KT, 128, BATCH)
    mu_rep = np.tile(mu[0], NS)                          # (M,)
    logstd_rep = np.tile(logstd, NS)                     # (M,)
    els_rep = np.exp(logstd_rep).astype(np.float32)      # (M,)

    in_maps = []
    for c in range(NCORES):
        rows = slice(c * RPC, (c + 1) * RPC)
        bt3 = np.ascontiguousarray(Bb[rows, :].T).reshape(KT, 128, RPC)
        tile_block = np.concatenate([bt3, epsT3], axis=2)   # (KT, 128, TW)
        arr = np.empty((128, NTC), dtype=bfl)
        arr[:, 0:TCOLS] = tile_block.transpose(1, 0, 2).reshape(128, TCOLS)
        arr[:, TCOLS:TCOLS + RPC] = np.broadcast_to(
            els_rep[rows].astype(bfl)[None, :], (128, RPC)
        )
        arr[:, TCOLS + RPC:NTC] = np.broadcast_to(
            mu_rep[rows].astype(bfl)[None, :], (128, RPC)
        )
        in_maps.append({"bte": arr})
    return in_maps, mu_rep, logstd_rep


def _run(mu, logstd, B, eps, batch_size, trace=False, trace_kwargs=None):
    mu = np.asarray(mu, dtype=np.float32)
    logstd = np.asarray(logstd, dtype=np.float32)
    B = np.asarray(B, dtype=np.float32)
    eps = np.asarray(eps, dtype=np.float32)
    b = int(batch_size)
    assert B.shape == (1, M, M) and eps.shape == (b, M, 1) and b == BATCH

    in_maps, mu_rep, logstd_rep = _prep_inputs(mu, logstd, B, eps)

    nc = _get_nc()
    kw = {}
    if trace:
        kw = dict(trace=True, trace_cores=list(range(NCORES)))
        if trace_kwargs:
            kw.update(trace_kwargs)
    res = bass_utils.run_bass_kernel_spmd(
        nc, in_maps, core_ids=list(range(NCORES)), **kw
    )

    samples_bm = np.concatenate(
        [np.asarray(res.results[c]["out"]).astype(np.float32) for c in range(NCORES)],
        axis=1,
    )  # (BATCH, M)
    samples = samples_bm.reshape(b, NS, Z)
    mu_out = np.broadcast_to(mu_rep[None, :], (b, M)).reshape(b, NS, Z).copy()
    logvar = (
        np.broadcast_to(2.0 * logstd_rep[None, :], (b, M)).reshape(b, NS, Z).copy()
    )
    return (mu_out, logvar, samples), res


def kernel(mu, logstd, B, eps, batch_size):
    outs, _ = _run(mu, logstd, B, eps, batch_size, trace=False)
    return outs


# revision 18
# speedup vs baseline: 1.1172x; 1.0648x over previous
"""TRN2 Bass kernel for nn_COV_75359496176097.

reference():
    B2 = B[0]                               # (8192, 8192)
    rn = sqrt(1 / sum(B2*B2, axis=1))       # row norms
    A  = rn * B2 * exp(tile(logstd, 64))[:, None]
    samples = tile(mu,64) + einsum('mk,bk->bm', A, eps[:,:,0])
    returns (mu_out, logvar, samples), each (128, 64, 128)

Strategy: shard B by rows across 8 cores (1024 rows each, no collectives),
all data in bf16 (tolerance 2e-2 >> bf16's ~3e-3 dot error) - halves HBM
traffic vs fp32: ~19.4MB/core. The whole packed input (64 k-tiles of
[B^T | eps^T] plus replicated exp(logstd)/mu rows, 148KB/partition) sits
resident in SBUF - no slot recycling. The DMA stream is issued as ~1.2MB
chunks alternating between the sync HWDGE queue and the gpsimd SWDGE
queue so two descriptor generators feed the 16 SDMA engines in parallel;
each chunk has its own completion semaphore (cross-queue completion is
not FIFO).

Per k-tile the PE runs two bf16 acc matmuls (eps^T stationary, B^T
moving, PSUM-accumulated). Row norms: DVE squares low columns, ACT high
columns (fused 2-tile ops via 3D strided APs to amortize per-op fixed
cost); for 'paired' pairs DVE also sums the two square-tiles so the
ones-stationary norm matmul runs once per pair instead of per tile -
trading cheap DVE adds for expensive PE moving columns. The pairing
fraction balances PE vs DVE vs ACT, all under the DMA roofline. The last
pair is unpaired and split per-tile so only tile 63's square chain
remains after the final chunk lands.

Epilogue: out = (acc*els) * sqrt(1/nrm) + mu. DVE reciprocal_approx_fast
on PSUM nrm (fp32), ACT sqrt -> bf16, then bf16 mul/add at DVE 2x rate;
bf16 output DMA per column half. Square and Sqrt share one ACT table set
(loaded once by a dummy sqrt during warmup). A 56-matmul warmup keeps
the PE HAM clock monitor warm until tile 0 lands.

Raw Bass (not Tile): one semaphore wait per instruction; consecutive
standalone waits AND together. Norm matmuls run one pair behind the
square producers (4 sq buffers -> 3 pairs of slack) so DVE/ACT never
stall the PE.
"""

import sys
from contextlib import ExitStack

if "/opt/trn_rl_repo" not in sys.path:
    sys.path.insert(0, "/opt/trn_rl_repo")

import ml_dtypes
import numpy as np

import concourse.bacc as bacc
import concourse.mybir as mybir
from concourse import bass_utils

Z = 128
NS = 64
M = Z * NS          # 8192
BATCH = 128
NCORES = 8
RPC = M // NCORES   # 1024 rows of B per core
KT = M // 128       # 64 k-tiles
TW = RPC + BATCH    # 1152 packed tile width (B^T cols | eps^T cols)
TCOLS = KT * TW     # 73728 tile columns
NTC = TCOLS + 2 * RPC  # + els, mu replicated rows -> 75776 total sbuf cols
NPAIR = KT // 2
WD_P = 640          # DVE/ACT column split for paired pairs
WD_U = 512          # and for unpaired (must be 512: single producer per half)
NSQ = 4             # square-buffer slots
NWARM = 44

# DMA chunks, in tiles. All issued in order on the sync HWDGE ring
# (gpsimd SWDGE adds a ~6.5us DRAIN per DMA and a ~5us preamble barrier -
# measured strictly worse). Each DMA's completion-semaphore descriptor
# stalls the SDMA ring ~0.9us (write-receipt ordering), so chunks are as
# big as consumer pacing allows. The els/mu constants sit INSIDE the
# packed layout between tiles 15 and 16, so they ride chunk (8,16) with
# no extra DMA; their arrival is implied by any later chunk's semaphore
# (ring FIFO order).
CHUNKS = [
    (0, 2), (2, 4), (4, 8),
    (8, 16), (16, 24), (24, 32), (32, 40), (40, 48), (48, 56),
    (56, 60), (60, 62), (62, 63), (63, 64),
]
NDMA = len(CHUNKS)
EM_AT = 16          # els/mu columns sit just before tile EM_AT's columns


def _toff(t):
    """column offset of tile t in the packed layout"""
    return t * TW + (2 * RPC if t >= EM_AT else 0)


def _tile_dma_idx(t):
    for i, ch in enumerate(CHUNKS):
        if ch[0] <= t < ch[1]:
            return i
    raise AssertionError(t)


def _same_chunk(q):
    return _tile_dma_idx(2 * q) == _tile_dma_idx(2 * q + 1)


def _paired(q):
    return _same_chunk(q)


f32 = mybir.dt.float32
bf16 = mybir.dt.bfloat16

_nc_cache = {}


def _build():
    nc = bacc.Bacc("TRN2", debug=False)

    bte_d = nc.dram_tensor("bte", (128, NTC), bf16, kind="ExternalInput")
    out_d = nc.dram_tensor("out", (BATCH, RPC), bf16, kind="ExternalOutput")

    # per-pair semaphore thresholds, precomputed (python-side counters)
    d0c = d1c = actc = 0
    thr = {}  # q -> dict with thresholds
    for q in range(NPAIR):
        if _paired(q):
            d0c += 1
            actc += 1
            d1c += 1
            thr[q] = dict(p=True, h0=d0c, h1=d1c, act=actc)
        elif _same_chunk(q):
            d0c += 1
            d1c += 1
            thr[q] = dict(p=False, eh0=d0c, eh1=d1c, oh0=d0c, oh1=d1c)
        else:
            d0c += 2
            d1c += 2
            thr[q] = dict(p=False, eh0=d0c - 1, eh1=d1c - 1, oh0=d0c, oh1=d1c)

    with ExitStack() as ctx:
        e = ctx.enter_context
        bte = e(nc.sbuf_tensor("bte_sb", [128, NTC], bf16))
        # slot s holds pair q=s (mod NSQ): [:, 0, :] even tile, [:, 1, :] odd
        sq2 = [e(nc.sbuf_tensor(f"sq2_{i}", [128, 2, RPC], bf16)) for i in range(NSQ)]
        ones = e(nc.sbuf_tensor("ones", [128, 128], bf16))
        dummy = e(nc.sbuf_tensor("dmy_sb", [128, 1], f32))
        scale = e(nc.sbuf_tensor("scale", [128, RPC], bf16))
        rr = e(nc.sbuf_tensor("rr", [128, RPC], f32))
        rs = e(nc.sbuf_tensor("rs", [128, RPC], bf16))
        out_sb = e(nc.sbuf_tensor("out_sb", [128, RPC], bf16))
        acc = e(nc.psum_tensor([128, RPC], f32))
        nrm = e(nc.psum_tensor([128, RPC], f32))
        warm_ps = e(nc.psum_tensor([128, 512], f32))

        s_dma = [e(nc.semaphore(name=f"s_dma{i}")) for i in range(NDMA)]
        s_wm = e(nc.semaphore(name="s_wm"))
        s_act = e(nc.semaphore(name="s_act"))
        s_d0 = e(nc.semaphore(name="s_d0"))
        s_d1 = e(nc.semaphore(name="s_d1"))
        s_nm = e(nc.semaphore(name="s_nm"))
        s_acc = e(nc.semaphore(name="s_acc"))
        s_r = e(nc.semaphore(name="s_r"))
        s_x = e(nc.semaphore(name="s_x"))
        s_out = e(nc.semaphore(name="s_out"))
        s_od = e(nc.semaphore(name="s_od"))

        block = e(nc.Block())

        H = (slice(0, 512), slice(512, RPC))

        def bslice(t, a, b):
            return bte[:, _toff(t) + a:_toff(t) + b]

        def bpair(q, a, b):
            """[128, 2, b-a] view of cols [a:b) of tiles (2q, 2q+1)."""
            te = 2 * q
            assert _toff(te + 1) - _toff(te) == TW  # never spans the els/mu gap
            return bte[:, _toff(te):_toff(te) + 2 * TW].rearrange(
                "p (t w) -> p t w", t=2
            )[:, :, a:b]

        def chunk_cols(ch):
            return _toff(ch[0]), _toff(ch[1]) if ch[1] < KT else NTC

        @block.sync
        def _(sync):
            for i, ch in enumerate(CHUNKS):
                c0, c1 = chunk_cols(ch)
                sync.dma_start(bte[:, c0:c1], bte_d.ap()[:, c0:c1]).then_inc(
                    s_dma[i], 16
                )
            for h in range(2):
                sync.wait_ge(s_out, h + 1)
                sync.dma_start(out_d.ap()[:, H[h]], out_sb[:, H[h]]).then_inc(
                    s_od, 16
                )
            sync.wait_ge(s_od, 32)
            sync.nop()

        @block.tensor
        def _(tensor):
            # warmup: pin the PE HAM activity monitor warm until tile 0 lands
            tensor.wait_ge(s_wm, 1)
            for _ in range(NWARM):
                nc.tensor.matmul(
                    warm_ps[:, 0:128], ones[:], ones[:], start=True, stop=True
                )

            def norm_mms(q, sub="all"):
                # sub: 'all' (normal, one pair behind), 'e'/'o' for the
                # split last pair
                s = q % NSQ
                tq = thr[q]
                if tq["p"]:
                    qst, qsp = q == 0, q == NPAIR - 1
                    tensor.wait_ge(s_d0, tq["h0"])
                    nc.tensor.matmul(
                        nrm[:, H[0]], ones[:], sq2[s][:, 0, H[0]],
                        start=qst, stop=qsp,
                    )
                    tensor.wait_ge(s_d1, tq["h1"])
                    nc.tensor.matmul(
                        nrm[:, H[1]], ones[:], sq2[s][:, 0, H[1]],
                        start=qst, stop=qsp,
                    ).then_inc(s_nm, 1)
                    return
                tiles = {"all": (0, 1), "e": (0,), "o": (1,)}[sub]
                for ti in tiles:
                    qst = q == 0 and ti == 0
                    qsp = q == NPAIR - 1 and ti == 1
                    tensor.wait_ge(s_d0, tq["eh0"] if ti == 0 else tq["oh0"])
                    nc.tensor.matmul(
                        nrm[:, H[0]], ones[:], sq2[s][:, ti, H[0]],
                        start=qst, stop=qsp,
                    )
                    tensor.wait_ge(s_d1, tq["eh1"] if ti == 0 else tq["oh1"])
                    ins = nc.tensor.matmul(
                        nrm[:, H[1]], ones[:], sq2[s][:, ti, H[1]],
                        start=qst, stop=qsp,
                    )
                    # exactly one s_nm inc per pair-slot: the final sub-call
                    if ti == tiles[-1] and sub != "e":
                        ins.then_inc(s_nm, 1)

            last_q = NPAIR - 1
            last_split = not _same_chunk(last_q)
            seen = -1
            for t in range(KT):
                di = _tile_dma_idx(t)
                if di > seen:
                    if last_split and t == KT - 1:
                        # run the last pair's even-tile norm matmuls before
                        # stalling on the final chunk
                        norm_mms(last_q, sub="e")
                    tensor.wait_ge(s_dma[di], 16)
                    seen = di
                st, sp = t == 0, t == KT - 1
                eps_v = bslice(t, RPC, TW)
                for h in range(2):
                    ins = nc.tensor.matmul(
                        acc[:, H[h]], eps_v, bslice(t, h * 512, (h + 1) * 512),
                        start=st, stop=sp,
                    )
                if sp:
                    ins.then_inc(s_acc, 1)
                if t % 2 == 1 and t >= 3:
                    norm_mms(t // 2 - 1)
            norm_mms(last_q, sub="o" if last_split else "all")

        @block.scalar
        def _(scalar):
            # dummy sqrt first: loads the sqrt_and_others ACT table set
            # (which also contains Square) once, during the DMA fill
            scalar.wait_ge(s_wm, 1)
            nc.scalar.sqrt(dummy[:], ones[:, 0:1])
            seen = -1
            for q in range(NPAIR):
                s = q % NSQ
                tq = thr[q]
                wd = WD_P if tq["p"] else WD_U
                die = _tile_dma_idx(2 * q)
                dio = _tile_dma_idx(2 * q + 1)
                if die > seen:
                    scalar.wait_ge(s_dma[die], 16)
                    seen = die
                if q >= NSQ:
                    scalar.wait_ge(s_nm, q - (NSQ - 1))
                if dio == die:
                    ins = nc.scalar.square(
                        sq2[s][:, :, wd:RPC], bpair(q, wd, RPC)
                    )
                    ins.then_inc(s_act if tq["p"] else s_d1, 1)
                else:
                    nc.scalar.square(
                        sq2[s][:, 0, wd:RPC], bslice(2 * q, wd, RPC)
                    ).then_inc(s_d1, 1)
                    scalar.wait_ge(s_dma[dio], 16)
                    seen = dio
                    nc.scalar.square(
                        sq2[s][:, 1, wd:RPC], bslice(2 * q + 1, wd, RPC)
                    ).then_inc(s_d1, 1)
            for h in range(2):
                scalar.wait_ge(s_r, h + 1)
                nc.scalar.sqrt(rs[:, H[h]], rr[:, H[h]]).then_inc(s_x, 1)

        @block.vector
        def _(vector):
            nc.vector.memset(ones[:], 1.0).then_inc(s_wm, 1)
            seen = -1
            for q in range(NPAIR):
                s = q % NSQ
                tq = thr[q]
                wd = WD_P if tq["p"] else WD_U
                die = _tile_dma_idx(2 * q)
                dio = _tile_dma_idx(2 * q + 1)
                if die > seen:
                    vector.wait_ge(s_dma[die], 16)
                    seen = die
                if q >= NSQ:
                    vector.wait_ge(s_nm, q - (NSQ - 1))
                if dio == die:
                    ins = nc.vector.tensor_mul(
                        sq2[s][:, :, 0:wd], bpair(q, 0, wd), bpair(q, 0, wd)
                    )
                    if not tq["p"]:
                        ins.then_inc(s_d0, 1)
                else:
                    be = bslice(2 * q, 0, wd)
                    nc.vector.tensor_mul(
                        sq2[s][:, 0, 0:wd], be, be
                    ).then_inc(s_d0, 1)
                    vector.wait_ge(s_dma[dio], 16)
                    seen = dio
                    bo = bslice(2 * q + 1, 0, wd)
                    nc.vector.tensor_mul(
                        sq2[s][:, 1, 0:wd], bo, bo
                    ).then_inc(s_d0, 1)
                if tq["p"]:
                    nc.vector.tensor_add(
                        sq2[s][:, 0, H[0]], sq2[s][:, 0, H[0]], sq2[s][:, 1, H[0]]
                    ).then_inc(s_d0, 1)
                    vector.wait_ge(s_act, tq["act"])
                    nc.vector.tensor_add(
                        sq2[s][:, 0, H[1]], sq2[s][:, 0, H[1]], sq2[s][:, 1, H[1]]
                    ).then_inc(s_d1, 1)

            # epilogue: out = (acc*els) * sqrt(1/nrm) + mu, by column halves
            els = bte[:, EM_AT * TW:EM_AT * TW + RPC]
            mu_v = bte[:, EM_AT * TW + RPC:EM_AT * TW + 2 * RPC]
            # els/mu arrival is implied by s_acc (tile 63's chunk postdates
            # chunk (8,16), which carries them, on the FIFO ring)
            vector.wait_ge(s_acc, 1)
            for h in range(2):
                nc.vector.tensor_mul(scale[:, H[h]], acc[:, H[h]], els[:, H[h]])
            vector.wait_ge(s_nm, NPAIR)
            for h in range(2):
                nc.vector.reciprocal_approx_fast(
                    out=rr[:, H[h]], in_=nrm[:, H[h]]
                ).then_inc(s_r, 1)
            for h in range(2):
                vector.wait_ge(s_x, h + 1)
                nc.vector.tensor_mul(out_sb[:, H[h]], scale[:, H[h]], rs[:, H[h]])
                nc.vector.tensor_add(
                    out_sb[:, H[h]], out_sb[:, H[h]], mu_v[:, H[h]]
                ).then_inc(s_out, 1)

    nc.compile()
    return nc


def _get_nc():
    if "nc" not in _nc_cache:
        _nc_cache["nc"] = _build()
    return _nc_cache["nc"]


def _prep_inputs(mu, logstd, B, eps):
    bfl = ml_dtypes.bfloat16
    B2 = B[0]
    Bb = B2.astype(bfl)                                  # (M, M)
    epsT3 = np.ascontiguousarray(eps[:, :, 0].T).astype(bfl).reshape(KT, 128, BATCH)
    mu_rep = np.tile(mu[0], NS)                          # (M,)
    logstd_rep = np.tile(logstd, NS)                     # (M,)
    els_rep = np.exp(logstd_rep).astype(np.float32)      # (M,)

    in_maps = []
    for c in range(NCORES):
        rows = slice(c * RPC, (c + 1) * RPC)
        bt3 = np.ascontiguousarray(Bb[rows, :].T).reshape(KT, 128, RPC)
        tile_block = np.concatenate([bt3, epsT3], axis=2)   # (KT, 128, TW)
        packed = tile_block.transpose(1, 0, 2)      # (128, KT, TW)
        arr = np.empty((128, NTC), dtype=bfl)
        em0 = EM_AT * TW
        arr[:, 0:em0] = packed[:, 0:EM_AT].reshape(128, em0)
        arr[:, em0:em0 + RPC] = np.broadcast_to(
            els_rep[rows].astype(bfl)[None, :], (128, RPC)
        )
        arr[:, em0 + RPC:em0 + 2 * RPC] = np.broadcast_to(
            mu_rep[rows].astype(bfl)[None, :], (128, RPC)
        )
        arr[:, em0 + 2 * RPC:NTC] = packed[:, EM_AT:].reshape(
            128, (KT - EM_AT) * TW
        )
        in_maps.append({"bte": arr})
    return in_maps, mu_rep, logstd_rep


def _run(mu, logstd, B, eps, batch_size, trace=False, trace_kwargs=None):
    mu = np.asarray(mu, dtype=np.float32)
    logstd = np.asarray(logstd, dtype=np.float32)
    B = np.asarray(B, dtype=np.float32)
    eps = np.asarray(eps, dtype=np.float32)
    b = int(batch_size)
    assert B.shape == (1, M, M) and eps.shape == (b, M, 1) and b == BATCH

    in_maps, mu_rep, logstd_rep = _prep_inputs(mu, logstd, B, eps)

    nc = _get_nc()
    kw = {}
    if trace:
        kw = dict(trace=True, trace_cores=list(range(NCORES)))
        if trace_kwargs:
            kw.update(trace_kwargs)
    res = bass_utils.run_bass_kernel_spmd(
        nc, in_maps, core_ids=list(range(NCORES)), **kw
    )

    samples_bm = np.concatenate(
        [np.asarray(res.results[c]["out"]).astype(np.float32) for c in range(NCORES)],
        axis=1,
    )  # (BATCH, M)
    samples = samples_bm.reshape(b, NS, Z)
    mu_out = np.broadcast_to(mu_rep[None, :], (b, M)).reshape(b, NS, Z).copy()
    logvar = (
        np.broadcast_to(2.0 * logstd_rep[None, :], (b, M)).reshape(b, NS, Z).copy()
    )
    return (mu_out, logvar, samples), res


def kernel(mu, logstd, B, eps, batch_size):
    outs, _ = _run(mu, logstd, B, eps, batch_size, trace=False)
    return outs
